# revision 1
# baseline (speedup 1.0000x reference)
"""Chamfer distance kernel for Trainium2 (8 NeuronCores, data-parallel over batch).

Problem: xyz1, xyz2: [8, 8192, 3] fp32.  Per batch b:
  d[i,j] = |x_i|^2 + |y_j|^2 - 2 x_i.y_j
  dist1[i] = min_j d[i,j]; idx1[i] = argmin_j d[i,j]   (and symmetrically dist2/idx2)

Strategy (one batch per core).  The device only finds ARGMINS; the host
recomputes the dist values from the indices with the same fp32 formula
(trivial vectorized numpy), so all large scans run in fp16, where DVE
tensor ops get 2x/4x rate.  fp16 is safe here because e = -d is near zero
at the maxima, where fp16's absolute resolution beats the ~1e-6 fp32
cancellation noise already present in e.

  - PE computes NEGATED distances e = 2 x.y - sq1 - sq2 with a K=5 fp32
    matmul (lhsT rows [x0 x1 x2, 1, -sq1] stationary per 128-row tile;
    rhs rows [2y0 2y1 2y2, -sq2, 1] moving, 512-col chunks); min/argmin of
    d becomes max/argmax of e.  ACT downconverts PSUM -> fp16 SBUF E.
  - Row path per row-tile: fp16 pairwise tensor_max tree -> rowmax; 4x-rate
    tensor_scalar is_ge mask (in place over E); ONE batched xbar-transpose
    DMA flips all 64 128x128 mask blocks; PE contracts the j-partitions of
    the transposed mask against weights [jp, 1, g] (16 N=512 matmuls
    accumulating in one PSUM bank) -> per-row candidate sums; host decodes
    j* = 512*S_g + 128*sum(sub*CNT_sub) + S_jp and exactly recomputes the
    ~1% of rows with CNT != 1 (fp16 near-ties).
  - Col path: fp16 running tensor_max into ACC plus a first-winning-tile
    tracker it = max(it, t*(e >= acc')) in 2x/4x ops; GPSIMD
    partition_all_reduce(max) finishes across partitions with tie-break by
    maximizing -(128 t + p).
"""

import os
import numpy as np

B = 8
N = 8192  # rows per batch (xyz1 points)
M = 8192  # cols per batch (xyz2 points)
P = 128
CHUNK = 512
N_CORES = 8

_cache = {}


def build(n=N, m=M, n_cores=N_CORES):
    """Build the Bass program. Returns the compiled Bacc object."""
    import concourse.bacc as bacc
    import concourse.tile as tile
    import concourse.mybir as mybir
    from concourse.bass_isa import ReduceOp

    dt = mybir.dt
    Alu = mybir.AluOpType
    Act = mybir.ActivationFunctionType

    nt = n // P        # row tiles
    nch = m // CHUNK   # column chunks
    fin = 2048 if m % 2048 == 0 else m  # finals chunk width
    nq = m // fin

    nc = bacc.Bacc(
        "TRN2",
        target_bir_lowering=False,
        debug=False,
        enable_asserts=False,
        num_devices=n_cores,
    )

    xt = nc.dram_tensor("xt", [3, n], dt.float32, kind="ExternalInput").ap()
    yt = nc.dram_tensor("yt", [3, m], dt.float32, kind="ExternalInput").ap()
    rsum_d = nc.dram_tensor("rsum", [3, nt * 512], dt.float32, kind="ExternalOutput").ap()
    idx2_d = nc.dram_tensor("idx2", [1, m], dt.uint32, kind="ExternalOutput").ap()

    with tile.TileContext(nc) as tc, tc.tile_pool(name="persist", bufs=1) as pp:
        # ---- constants / prep ----
        lhs = pp.tile([5, n], dt.float32, name="lhs")
        rhs = pp.tile([5, m], dt.float32, name="rhs")
        ones3 = pp.tile([3, 1], dt.float32, name="ones3")
        piota_u = pp.tile([P, 1], dt.uint32, name="piota_u")
        npiota_f = pp.tile([P, 1], dt.float32, name="npiota_f")

        nc.vector.memset(ones3[:], 1.0)
        # piota_u[p, 0] = p ; npiota_f = -p
        nc.gpsimd.iota(piota_u[:], pattern=[[0, 1]], base=0, channel_multiplier=1)
        nc.vector.tensor_scalar_mul(npiota_f[:], piota_u[:], -1.0)

        # load points
        nc.sync.dma_start(lhs[0:3, :], xt[:, :])
        nc.sync.dma_start(rhs[0:3, :], yt[:, :])

        # squared norms via ones^T @ (pts^2).  Engine ops must start at
        # partition 0, so -sq goes to a base-0 scratch row, then SBUF->SBUF
        # DMA places it into partition 3/4 of lhs/rhs.
        with (
            tc.tile_pool(name="sq_pool", bufs=1) as sqp,
            tc.tile_pool(name="psum_sq", bufs=2, space="PSUM") as psum_sq,
        ):
            sq_tmp = sqp.tile([3, max(n, m)], dt.float32, name="sq_tmp")
            # one base-0 scratch row, reused sequentially: ones -> -sq1 -> -sq2
            row = sqp.tile([1, max(n, m)], dt.float32, name="row")
            nc.vector.memset(row[:], 1.0)
            nc.sync.dma_start(lhs[3:4, :], row[:, 0:n])
            nc.sync.dma_start(rhs[4:5, :], row[:, 0:m])
            nsq1 = row[:, 0:n]
            nsq2 = row[:, 0:m]

            wn = min(CHUNK, n)
            nc.scalar.activation(sq_tmp[:, 0:n], lhs[0:3, :], Act.Square)
            for c in range(n // wn):
                ps = psum_sq.tile([1, wn], dt.float32, tag="ps_sq")
                nc.tensor.matmul(
                    ps[:], ones3[:], sq_tmp[:, c * wn:(c + 1) * wn],
                    start=True, stop=True,
                )
                nc.scalar.activation(
                    nsq1[:, c * wn:(c + 1) * wn], ps[:], Act.Copy, scale=-1.0
                )
            nc.sync.dma_start(lhs[4:5, :], nsq1[:])
            wm = min(CHUNK, m)
            nc.scalar.activation(sq_tmp[:, 0:m], rhs[0:3, :], Act.Square)
            for c in range(m // wm):
                ps = psum_sq.tile([1, wm], dt.float32, tag="ps_sq")
                nc.tensor.matmul(
                    ps[:], ones3[:], sq_tmp[:, c * wm:(c + 1) * wm],
                    start=True, stop=True,
                )
                nc.scalar.activation(
                    nsq2[:, c * wm:(c + 1) * wm], ps[:], Act.Copy, scale=-1.0
                )
            # lhs rows: [x0 x1 x2, 1, -sq1]; rhs rows: [2y0 2y1 2y2, -sq2, 1]
            nc.sync.dma_start(rhs[3:4, :], nsq2[:])
        # scale y by 2 (after sq2 computed)
        nc.vector.tensor_scalar_mul(rhs[0:3, :], rhs[0:3, :], 2.0)

        # indicator weights: WJ[:, 3g:3g+3] = [jp_iota, 1, g] (fp16 exact)
        ngrp = (m // P) // 4
        wj = pp.tile([P, 3 * ngrp], dt.float16, name="wj")
        wj_scr = pp.tile([P, ngrp], dt.uint32, name="wj_scr")
        nc.gpsimd.iota(wj_scr[:], pattern=[[0, ngrp]], base=0, channel_multiplier=1)
        nc.vector.tensor_copy(wj[:, 0:3 * ngrp:3], wj_scr[:])   # jp
        nc.vector.memset(wj[:, 1:3 * ngrp:3], 1.0)              # ones
        nc.gpsimd.iota(wj_scr[:], pattern=[[1, ngrp]], base=0, channel_multiplier=0)
        nc.vector.tensor_copy(wj[:, 2:3 * ngrp:3], wj_scr[:])   # g

        # ---- persistent state ----
        acc = pp.tile([P, m], dt.float16, name="acc")      # running col max of e (fp16)
        it_t = pp.tile([P, m], dt.float16, name="it_t")    # first row-tile idx achieving acc

        nc.vector.memset(acc[:], -60000.0)
        nc.vector.memset(it_t[:], 0)

        # ---- main loop ----
        with (
            tc.tile_pool(name="psum_e", bufs=7, space="PSUM") as psum_e,
            tc.tile_pool(name="psum_idx", bufs=1, space="PSUM") as psum_idx,
            tc.tile_pool(name="e_pool", bufs=3) as e_pool,
            tc.tile_pool(name="cmp_pool", bufs=4) as cmp_pool,
            tc.tile_pool(name="tree_pool", bufs=2) as tree_pool,
            tc.tile_pool(name="mask_pool", bufs=2) as mask_pool,
            tc.tile_pool(name="drain_pool", bufs=2) as drain_pool,
        ):
            for t in range(nt):
                e_row = e_pool.tile([P, m], dt.float16, tag="e_row")
                for c in range(nch):
                    cs = slice(c * CHUNK, (c + 1) * CHUNK)
                    ps = psum_e.tile([P, CHUNK], dt.float32, tag="ps")
                    nc.tensor.matmul(
                        ps[:], lhs[:, t * P:(t + 1) * P], rhs[:, cs],
                        start=True, stop=True,
                    )
                    # row path raw data (SBUF copy)
                    nc.scalar.copy(e_row[:, cs], ps[:])
                    # col path (all fp16 SBUF, 2x/4x DVE modes): update
                    # running max, detect "this tile won" via e >= acc',
                    # record t.  t strictly increases, so it = max(it, t*cmp)
                    # keeps the winning row-tile.  Per-chunk cadence keeps
                    # the PE/ACT/DVE pipeline smooth (coarser spans stall PE).
                    nc.vector.tensor_max(acc[:, cs], acc[:, cs], e_row[:, cs])
                    if t > 0:
                        cmp = cmp_pool.tile([P, CHUNK], dt.float16, tag="cmp")
                        nc.vector.tensor_tensor(cmp[:], e_row[:, cs], acc[:, cs], op=Alu.is_ge)
                        nc.vector.tensor_scalar_mul(cmp[:], cmp[:], float(t))
                        nc.vector.tensor_max(it_t[:, cs], it_t[:, cs], cmp[:])
                # row path: fp16 pairwise-max tree -> rowmax, then a 4x-rate
                # is_ge mask (accum_out counts candidates), DMA-transposed so
                # the PE can extract the index as sum(mask*[jp,1,b]).
                scr = tree_pool.tile([P, m // 2], dt.float16, tag="scr")
                rmx = tree_pool.tile([P, 1], dt.float32, tag="rmx")
                h = m // 2
                nc.vector.tensor_max(scr[:, 0:h], e_row[:, 0:h], e_row[:, h:m])
                while h > 64:
                    nc.vector.tensor_max(
                        scr[:, 0:h // 2], scr[:, 0:h // 2], scr[:, h // 2:h]
                    )
                    h //= 2
                nc.vector.tensor_reduce(
                    rmx[:], scr[:, 0:h], axis=mybir.AxisListType.X, op=Alu.max
                )
                # mask overwrites e_row in place (last reader of e); one
                # batched xbar transpose gives maskt[:, b, :] = block b^T
                nc.vector.tensor_scalar(
                    e_row[:], e_row[:], rmx[:], None, op0=Alu.is_ge,
                )
                maskt = mask_pool.tile([P, m // P, P], dt.float16, tag="maskt")
                nc.sync.dma_start(maskt[:], e_row[:], transpose=True)
                # PE extracts Sum(mask*[jp,1,g]) per (sub, i): 16 matmuls of
                # N=512 (4 j-blocks each) accumulating in one PSUM bank.
                idx_ps = psum_idx.tile([3, 512], dt.float32, tag="idx_ps")
                for g in range(ngrp):
                    nc.tensor.matmul(
                        idx_ps[:], wj[:, 3 * g:3 * g + 3],
                        maskt[:, 4 * g:4 * g + 4, :],
                        start=(g == 0), stop=(g == ngrp - 1),
                    )
                dr = drain_pool.tile([3, 512], dt.float32, tag="dr")
                nc.scalar.copy(dr[:], idx_ps[:])
                nc.sync.dma_start(rsum_d[:, t * 512:(t + 1) * 512], dr[:])

        # ---- col outputs (chunked finals) ----
        with tc.tile_pool(name="fin_pool", bufs=1) as fp:
            for q in range(nq):
                qs = slice(q * fin, (q + 1) * fin)
                ar = fp.tile([P, fin], dt.float16, tag="ar")
                nc.gpsimd.partition_all_reduce(ar[:], acc[:, qs], P, ReduceOp.max)
                mq = fp.tile([P, fin], dt.uint8, tag="mq")
                nc.vector.tensor_tensor(mq[:], acc[:, qs], ar[:], op=Alu.is_equal)
                # ng = -(128*t + p) for candidates
                ng = fp.tile([P, fin], dt.float32, tag="ng")
                nc.vector.tensor_scalar(
                    ng[:], it_t[:, qs], -128.0, npiota_f[:],
                    op0=Alu.mult, op1=Alu.add,
                )
                sel = fp.tile([P, fin], dt.float32, tag="sel")
                nc.vector.memset(sel[:], -1e30)
                nc.vector.copy_predicated(sel[:], mq[:], ng[:])
                ar2 = fp.tile([P, fin], dt.float32, tag="ar2")
                nc.gpsimd.partition_all_reduce(ar2[:], sel[:], P, ReduceOp.max)
                # idx2 = -ar2
                i2f = fp.tile([1, fin], dt.float32, tag="i2f")
                nc.scalar.activation(i2f[:], ar2[0:1, :], Act.Copy, scale=-1.0)
                i2u = fp.tile([1, fin], dt.uint32, tag="i2u")
                nc.vector.tensor_copy(i2u[:], i2f[:])
                nc.sync.dma_start(idx2_d[:, qs], i2u[:])

    nc.compile()
    return nc


def _run(nc, xyz1, xyz2, n_cores, trace=False):
    from concourse import bass_utils

    in_maps = []
    for b in range(n_cores):
        in_maps.append({
            "xt": np.ascontiguousarray(xyz1[b].T).astype(np.float32),
            "yt": np.ascontiguousarray(xyz2[b].T).astype(np.float32),
        })
    res = bass_utils.run_bass_kernel_spmd(
        nc, in_maps, core_ids=list(range(n_cores)), trace=trace,
    )
    return res


def _assemble_idx1(rsum, x, y):
    """idx1 from the PE indicator sums (r[w, t, sub, p] for row i=128t+p:
    j = 512*S_g + 128*sum(sub*CNT_sub) + S_jp); rows with !=1 candidates get
    an exact fp32 recompute on the host (fp16 near-ties, ~1% of rows)."""
    nt = rsum.shape[1] // 512
    r = rsum.reshape(3, nt, 4, 128)
    s_jp = r[0].sum(1)
    cnt_sub = r[1]
    s_g = r[2].sum(1)
    s_sub = (cnt_sub * np.arange(4.0)[None, :, None]).sum(1)
    idx1 = np.rint(512.0 * s_g + 128.0 * s_sub + s_jp).reshape(-1).astype(np.int64)
    cnt = cnt_sub.sum(1).reshape(-1)
    bad = np.nonzero(cnt != 1.0)[0]
    if bad.size:
        x = x.astype(np.float32)
        y = y.astype(np.float32)
        sq1 = (x * x).sum(-1)
        sq2 = (y * y).sum(-1)
        d = sq1[bad, None] + sq2[None, :] - 2.0 * (x[bad] @ y.T)
        idx1[bad] = d.argmin(1)
    return idx1


def _host_dists(x, y, idx1, idx2):
    """Exact fp32 dists from device indices, same formula as the reference:
    d = sq1 + sq2 - 2 x.y  (device only finds argmins)."""
    sq1 = (x * x).sum(-1)                      # [n]
    sq2 = (y * y).sum(-1)                      # [m]
    g1 = y[idx1]                               # [n, 3]
    dist1 = sq1 + sq2[idx1] - 2.0 * (x * g1).sum(-1)
    g2 = x[idx2]                               # [m, 3]
    dist2 = sq2 + sq1[idx2] - 2.0 * (y * g2).sum(-1)
    return dist1.astype(np.float32), dist2.astype(np.float32)


def kernel(xyz1, xyz2, trace=False, _return_res=False):
    xyz1 = np.asarray(xyz1)
    xyz2 = np.asarray(xyz2)
    b, n, _ = xyz1.shape
    m = xyz2.shape[1]
    key = (n, m, b)
    if key not in _cache:
        _cache[key] = build(n=n, m=m, n_cores=b)
    nc = _cache[key]
    res = _run(nc, xyz1, xyz2, b, trace=trace)

    idx2 = np.stack([r["idx2"].reshape(-1) for r in res.results]).astype(np.int32)
    idx1 = np.stack([
        _assemble_idx1(r["rsum"], xyz1[bb], xyz2[bb])
        for bb, r in enumerate(res.results)
    ]).astype(np.int32)
    d1l, d2l = [], []
    for bb in range(b):
        d1, d2 = _host_dists(xyz1[bb].astype(np.float32), xyz2[bb].astype(np.float32),
                             idx1[bb], idx2[bb])
        d1l.append(d1)
        d2l.append(d2)
    out = (np.stack(d1l), np.stack(d2l), idx1, idx2)
    if _return_res:
        return out, res
    return out


if __name__ == "__main__":
    rng = np.random.default_rng(0)
    x = rng.standard_normal((8, N, 3), dtype=np.float32)
    y = rng.standard_normal((8, M, 3), dtype=np.float32)
    d1, d2, i1, i2 = kernel(x, y)
    print("ok", d1.shape, d2.shape, i1.shape, i2.shape)

